# revision 21
# baseline (speedup 1.0000x reference)
"""GCN block (3 layers) on 8 trn2 NeuronCores, data-parallel over batch.

Math: each layer is X' = (adj + I) @ leaky_relu(X @ W).
Let A = adj + I. Using (A @ H) @ W == A @ (H @ W), fold each layer's weight
into the previous layer's output so every layer is one big matmul against A:

    H0 = lrelu(X0 W0)                 (tiny, on-chip)
    G0 = H0 W1 ; Z1 = A G0 ; H1 = lrelu(Z1)
    G1 = H1 W2 ; Z2 = A G1 ; H2 = lrelu(Z2)
    G2 = H2     ; X3 = A G2           (final output)

Per core: 8 samples x 16 features = 128 = partition width. Layouts:
    T-layout  [c=(b,d), m]   (128 partitions, N free)
    N-layout  [m, c]         (m partitions, 128 free)
Big matmul: out = lhsT.T @ rhs with lhsT = G (N-layout, stationary) and
rhs = A^T tiles (streamed from HBM) -> Z^T in T-layout. The 16x16 weights
are expanded to 128x128 block-diagonal so the tiny matmuls run all 8
samples at once:  G = (H^T)^T @ W_blk  via lhsT = H^T tile (T-layout).
A^T = adj.T + I is built on the host (layout prep), streamed 3x per core.
"""

import numpy as np

N_FULL = 4096
D = 16
B_FULL = 64
NCORES = 8
B_CORE = B_FULL // NCORES  # 8
C = B_CORE * D  # 128 partitions
P = 128
NEG_SLOPE = 0.2

_CACHE = {}


USE_ACT_LRELU = False


def _leaky(nc, dest, ps, pool, width):
    """dest = leaky_relu(ps), PSUM -> SBUF."""
    import concourse.mybir as mybir

    if USE_ACT_LRELU:
        # single ScalarEngine op (CoreSim lacks Lrelu; validated on HW)
        nc.scalar.activation(
            dest, ps[:], mybir.ActivationFunctionType.Lrelu, alpha=NEG_SLOPE
        )
    else:
        # DVE fallback: one instruction may read PSUM only once
        tmp = pool.tile([P, width], mybir.dt.float32, tag="lk")
        nc.vector.tensor_scalar_mul(tmp[:], ps[:], NEG_SLOPE)
        nc.vector.tensor_max(dest, ps[:], tmp[:])


def _build_nc(n, free, dt_big_name):
    """Build the Bass module (per-core program). Cached per config.

    dt_big_name: "float32" | "bfloat16" | "float32r".
      bfloat16: A^T/G/H^T/X^T/W stored bf16 (half DMA, full-rate PE).
      float32r: f32 storage, matmuls bitcast to fp32r (full-rate PE at
                free>=256, fp32 DMA cost, ~tf32 matmul precision).
    """
    import concourse.bass as bass
    import concourse.mybir as mybir
    import concourse.tile as tile
    from concourse import bacc

    f32 = mybir.dt.float32
    r32 = dt_big_name == "float32r"
    dt_st = f32 if r32 else getattr(mybir.dt, dt_big_name)  # storage dtype
    dt_act = dt_st  # activations/weights storage

    def mm(ap):
        # matmul-operand view: bitcast to fp32r in r32 mode
        return ap.bitcast(mybir.dt.float32r) if r32 else ap

    nt = n // P        # number of 128-row m-tiles
    nch = n // free    # output column chunks
    tpb = 2              # m-tiles per A^T panel
    mb = n // (tpb * P)  # number of A^T row panels

    nc = bacc.Bacc(
        "TRN2", target_bir_lowering=False, debug=False, num_devices=NCORES
    )
    xt_h = nc.dram_tensor("xt", [C, n], dt_act, kind="ExternalInput")
    at_h = nc.dram_tensor("at", [n, n], dt_st, kind="ExternalInput")
    w_h = nc.dram_tensor("wt", [4, P, P], dt_act, kind="ExternalInput")
    out_h = nc.dram_tensor("out", [C, n], f32, kind="ExternalOutput")

    cache_k = 4 if mybir.dt.size(dt_st) == 2 else 0  # A^T panels pinned in SBUF
    cache_k = min(cache_k, mb)
    at_bufs = 4 if mybir.dt.size(dt_st) == 2 else 3

    def panel_src(i):
        return at_h[i * tpb * P:(i + 1) * tpb * P, :].rearrange(
            "(t p) n -> p t n", p=P
        )

    with tile.TileContext(nc) as tc:
        with (
            tc.tile_pool(name="const", bufs=1) as constp,
            tc.tile_pool(name="xtp", bufs=4) as xtp,
            tc.tile_pool(name="ht", bufs=2) as htp,
            tc.tile_pool(name="g", bufs=2) as gp,
            tc.tile_pool(name="ats", bufs=at_bufs) as atp,
            tc.tile_pool(name="outp", bufs=2) as outp,
            tc.tile_pool(name="lk", bufs=2) as lkp,
            tc.tile_pool(name="ps", bufs=8, space="PSUM") as psp,
        ):
            w_sb = constp.tile([P, 4, P], dt_act)
            nc.scalar.dma_start(w_sb[:], w_h[:].rearrange("w p q -> p w q"))

            # resident A^T panels: filled during layer 0, reused by layers 1-2
            at_cache = [
                constp.tile([P, tpb, n], dt_st, name=f"atc{i}")
                for i in range(cache_k)
            ]

            # H0^T = lrelu(W0_blk.T @ X0^T)  (T-layout)
            ht_cur = htp.tile([C, n], dt_act)
            for ch in range(nch):
                xtc = xtp.tile([C, free], dt_act, tag="xtc")
                nc.scalar.dma_start(xtc[:], xt_h[:, ch * free:(ch + 1) * free])
                ps = psp.tile([P, free], f32, tag="ps")
                nc.tensor.matmul(
                    ps[:], mm(w_sb[:, 0, :]), mm(xtc[:]), start=True, stop=True
                )
                _leaky(nc, ht_cur[:, ch * free:(ch + 1) * free], ps, lkp, free)

            for layer in range(3):
                w_idx = layer + 1  # W1_blk, W2_blk, I128
                # tiny: G = (H^T)^T @ W_blk  (N-layout)
                g_sb = gp.tile([P, n], dt_st)
                for mt in range(nt):
                    psg = psp.tile([P, P], f32, tag="ps")
                    nc.tensor.matmul(
                        psg[:],
                        mm(ht_cur[:, mt * P:(mt + 1) * P]),
                        mm(w_sb[:, w_idx, :]),
                        start=True,
                        stop=True,
                    )
                    nc.vector.tensor_copy(g_sb[:, mt * P:(mt + 1) * P], psg[:])

                # big: Z^T = sum_m G[m,:].T @ A^T[m, :]
                # m-outer: stream full row-panels of A^T (fat contiguous
                # DMA runs); all nch psum banks accumulate in parallel;
                # one stationary G tile serves nch matmuls per t-step.
                last = layer == 2
                dest = None if last else htp.tile([C, n], dt_act, name="htn")
                ps_list = [
                    psp.tile([P, free], f32, tag="ps", name=f"psc{i}")
                    for i in range(nch)
                ]
                # streamed panels with cached panels interleaved so the
                # stream prefetch catches up during DMA-free cached phases;
                # final layer runs cached panels first so the kernel ENDS
                # on streamed panels (DMA busy to the last matmul)
                streamed = list(range(cache_k, mb))
                if last:
                    order = list(range(cache_k)) + streamed
                else:
                    order = streamed[:]
                    for i in range(cache_k):
                        pos = (i + 1) * mb // (cache_k + 1)
                        order.insert(min(pos, len(order)), i)
                for oi, mbx in enumerate(order):
                    if mbx < cache_k:
                        att = at_cache[mbx]
                        if layer == 0:
                            nc.sync.dma_start(att[:], panel_src(mbx))
                    else:
                        att = atp.tile([P, tpb, n], dt_st, tag="att")
                        nc.sync.dma_start(att[:], panel_src(mbx))
                    for t in range(tpb):
                        mt = mbx * tpb + t
                        for ncx in range(nch):
                            nc.tensor.matmul(
                                ps_list[ncx][:],
                                mm(g_sb[:, mt * P:(mt + 1) * P]),
                                mm(att[:, t, ncx * free:(ncx + 1) * free]),
                                start=(oi == 0 and t == 0),
                                stop=(oi == len(order) - 1 and t == tpb - 1),
                            )
                for ncx in range(nch):
                    if last:
                        oc = outp.tile([C, free], f32, tag="oc")
                        nc.vector.tensor_copy(oc[:], ps_list[ncx][:])
                        nc.scalar.dma_start(
                            out_h[:, ncx * free:(ncx + 1) * free], oc[:]
                        )
                    else:
                        _leaky(
                            nc,
                            dest[:, ncx * free:(ncx + 1) * free],
                            ps_list[ncx],
                            lkp,
                            free,
                        )
                ht_cur = dest

    nc.compile()
    return nc


def _get_nc(n, free, dt_big_name):
    key = (n, free, dt_big_name)
    if key not in _CACHE:
        _CACHE[key] = _build_nc(n, free, dt_big_name)
    return _CACHE[key]


def _block_diag(w, reps):
    """(D,D) -> (reps*D, reps*D) block diagonal, f32."""
    d = w.shape[0]
    out = np.zeros((reps * d, reps * d), dtype=np.float32)
    for b in range(reps):
        out[b * d:(b + 1) * d, b * d:(b + 1) * d] = w
    return out


def prepare_inputs(x, adj, Identity, W0, W1, W2, n=N_FULL, dt_big_name="float32"):
    """Host-side layout prep. Returns per-core input maps."""
    b_full = x.shape[0]
    b_core = b_full // NCORES
    c = b_core * D

    if dt_big_name == "bfloat16":
        import ml_dtypes
        np_st = ml_dtypes.bfloat16
    elif dt_big_name == "float16":
        np_st = np.float16
    else:
        np_st = np.float32

    at = np.ascontiguousarray(
        adj.T.astype(np.float32) + Identity.T.astype(np.float32)
    ).astype(np_st)

    reps = c // D
    w_all = np.stack(
        [
            _block_diag(np.asarray(W0, np.float32), reps),
            _block_diag(np.asarray(W1, np.float32), reps),
            _block_diag(np.asarray(W2, np.float32), reps),
            np.eye(c, dtype=np.float32),
        ]
    ).astype(np_st)

    # xt[core][b*D+d, m] = x[core*b_core + b, m, d]
    xf = np.asarray(x, np.float32)
    in_maps = []
    for core in range(NCORES):
        xs = xf[core * b_core:(core + 1) * b_core]      # (b_core, n, D)
        xt = np.ascontiguousarray(xs.transpose(0, 2, 1).reshape(c, n)).astype(np_st)
        in_maps.append({"xt": xt, "at": at, "wt": w_all})
    return in_maps


def gather_output(results, n=N_FULL, b_full=B_FULL):
    b_core = b_full // NCORES
    c = b_core * D
    out = np.empty((b_full, n, D), dtype=np.float32)
    for core in range(NCORES):
        oc = np.asarray(results[core]["out"], np.float32).reshape(b_core, D, n)
        out[core * b_core:(core + 1) * b_core] = oc.transpose(0, 2, 1)
    return out


def run(x, adj, Identity, W0, W1, W2, n=N_FULL, free=512,
        dt_big_name="float16", trace=False):
    from concourse.bass_utils import run_bass_kernel_spmd

    nc = _get_nc(n, free, dt_big_name)
    in_maps = prepare_inputs(x, adj, Identity, W0, W1, W2, n, dt_big_name)
    core_ids = list(range(NCORES))
    res = run_bass_kernel_spmd(nc, in_maps, core_ids, trace=trace)
    out = gather_output(res.results, n, x.shape[0])
    return out, res


def kernel(x, adj, Identity, W0, W1, W2):
    out, _ = run(x, adj, Identity, W0, W1, W2)
    return out


# revision 22
# speedup vs baseline: 1.0752x; 1.0752x over previous
"""GCN block (3 layers) on 8 trn2 NeuronCores, data-parallel over batch.

Math: each layer is X' = (adj + I) @ leaky_relu(X @ W).
Let A = adj + I. Using (A @ H) @ W == A @ (H @ W), fold each layer's weight
into the previous layer's output so every layer is one big matmul against A:

    H0 = lrelu(X0 W0)                 (tiny, on-chip)
    G0 = H0 W1 ; Z1 = A G0 ; H1 = lrelu(Z1)
    G1 = H1 W2 ; Z2 = A G1 ; H2 = lrelu(Z2)
    G2 = H2     ; X3 = A G2           (final output)

Per core: 8 samples x 16 features = 128 = partition width. Layouts:
    T-layout  [c=(b,d), m]   (128 partitions, N free)
    N-layout  [m, c]         (m partitions, 128 free)
Big matmul: out = lhsT.T @ rhs with lhsT = G (N-layout, stationary) and
rhs = A^T tiles (streamed from HBM) -> Z^T in T-layout. The 16x16 weights
are expanded to 128x128 block-diagonal so the tiny matmuls run all 8
samples at once:  G = (H^T)^T @ W_blk  via lhsT = H^T tile (T-layout).
A^T = adj.T + I is built on the host (layout prep), streamed 3x per core.
"""

import numpy as np

N_FULL = 4096
D = 16
B_FULL = 64
NCORES = 8
B_CORE = B_FULL // NCORES  # 8
C = B_CORE * D  # 128 partitions
P = 128
NEG_SLOPE = 0.2

_CACHE = {}


def _leaky(nc, dest, ps, pool, width):
    """dest = leaky_relu(ps) = 0.2*ps + 0.8*relu(ps), PSUM -> SBUF.

    Split across engines: ACT computes t = relu(0.8*ps) (scale commutes
    with relu), DVE computes dest = ps*0.2 + t. Each instruction reads
    PSUM at most once (HW constraint).
    """
    import concourse.mybir as mybir

    t = pool.tile([P, width], mybir.dt.float32, tag="lk")
    nc.scalar.activation(
        t[:], ps[:], mybir.ActivationFunctionType.Relu, scale=1.0 - NEG_SLOPE
    )
    nc.vector.scalar_tensor_tensor(
        dest, ps[:], NEG_SLOPE, t[:], mybir.AluOpType.mult, mybir.AluOpType.add
    )


def _build_nc(n, free, dt_big_name):
    """Build the Bass module (per-core program). Cached per config.

    dt_big_name: "float32" | "bfloat16" | "float32r".
      bfloat16: A^T/G/H^T/X^T/W stored bf16 (half DMA, full-rate PE).
      float32r: f32 storage, matmuls bitcast to fp32r (full-rate PE at
                free>=256, fp32 DMA cost, ~tf32 matmul precision).
    """
    import concourse.bass as bass
    import concourse.mybir as mybir
    import concourse.tile as tile
    from concourse import bacc

    f32 = mybir.dt.float32
    r32 = dt_big_name == "float32r"
    dt_st = f32 if r32 else getattr(mybir.dt, dt_big_name)  # storage dtype
    dt_act = dt_st  # activations/weights storage

    def mm(ap):
        # matmul-operand view: bitcast to fp32r in r32 mode
        return ap.bitcast(mybir.dt.float32r) if r32 else ap

    nt = n // P        # number of 128-row m-tiles
    nch = n // free    # output column chunks
    tpb = 2              # m-tiles per A^T panel
    mb = n // (tpb * P)  # number of A^T row panels

    nc = bacc.Bacc(
        "TRN2", target_bir_lowering=False, debug=False, num_devices=NCORES
    )
    xt_h = nc.dram_tensor("xt", [C, n], dt_act, kind="ExternalInput")
    at_h = nc.dram_tensor("at", [n, n], dt_st, kind="ExternalInput")
    w_h = nc.dram_tensor("wt", [4, P, P], dt_act, kind="ExternalInput")
    out_h = nc.dram_tensor("out", [C, n], f32, kind="ExternalOutput")

    cache_k = 5 if mybir.dt.size(dt_st) == 2 else 0  # A^T panels pinned in SBUF
    cache_k = min(cache_k, mb)
    at_bufs = 4 if mybir.dt.size(dt_st) == 2 else 3

    def panel_src(i):
        return at_h[i * tpb * P:(i + 1) * tpb * P, :].rearrange(
            "(t p) n -> p t n", p=P
        )

    with tile.TileContext(nc) as tc:
        with (
            tc.tile_pool(name="const", bufs=1) as constp,
            tc.tile_pool(name="xtp", bufs=2) as xtp,
            tc.tile_pool(name="ht", bufs=2) as htp,
            tc.tile_pool(name="g", bufs=2) as gp,
            tc.tile_pool(name="ats", bufs=at_bufs) as atp,
            tc.tile_pool(name="outp", bufs=2) as outp,
            tc.tile_pool(name="lk", bufs=2) as lkp,
            tc.tile_pool(name="ps", bufs=8, space="PSUM") as psp,
        ):
            w_sb = constp.tile([P, 4, P], dt_act)
            nc.scalar.dma_start(w_sb[:], w_h[:].rearrange("w p q -> p w q"))

            # resident A^T panels: filled during layer 0, reused by layers 1-2
            at_cache = [
                constp.tile([P, tpb, n], dt_st, name=f"atc{i}")
                for i in range(cache_k)
            ]

            # H0^T = lrelu(W0_blk.T @ X0^T)  (T-layout)
            ht_cur = htp.tile([C, n], dt_act)
            for ch in range(nch):
                xtc = xtp.tile([C, free], dt_act, tag="xtc")
                nc.scalar.dma_start(xtc[:], xt_h[:, ch * free:(ch + 1) * free])
                ps = psp.tile([P, free], f32, tag="ps")
                nc.tensor.matmul(
                    ps[:], mm(w_sb[:, 0, :]), mm(xtc[:]), start=True, stop=True
                )
                _leaky(nc, ht_cur[:, ch * free:(ch + 1) * free], ps, lkp, free)

            for layer in range(3):
                w_idx = layer + 1  # W1_blk, W2_blk, I128
                # tiny: G = (H^T)^T @ W_blk  (N-layout)
                g_sb = gp.tile([P, n], dt_st)
                for mt in range(nt):
                    psg = psp.tile([P, P], f32, tag="ps")
                    nc.tensor.matmul(
                        psg[:],
                        mm(ht_cur[:, mt * P:(mt + 1) * P]),
                        mm(w_sb[:, w_idx, :]),
                        start=True,
                        stop=True,
                    )
                    nc.vector.tensor_copy(g_sb[:, mt * P:(mt + 1) * P], psg[:])

                # big: Z^T = sum_m G[m,:].T @ A^T[m, :]
                # m-outer: stream full row-panels of A^T (fat contiguous
                # DMA runs); all nch psum banks accumulate in parallel;
                # one stationary G tile serves nch matmuls per t-step.
                last = layer == 2
                dest = None if last else htp.tile([C, n], dt_act, name="htn")
                ps_list = [
                    psp.tile([P, free], f32, tag="ps", name=f"psc{i}")
                    for i in range(nch)
                ]
                # streamed panels with cached panels interleaved so the
                # stream prefetch catches up during DMA-free cached phases;
                # final layer runs cached panels first so the kernel ENDS
                # on streamed panels (DMA busy to the last matmul)
                streamed = list(range(cache_k, mb))
                if last:
                    order = list(range(cache_k)) + streamed
                else:
                    order = streamed[:]
                    for i in range(cache_k):
                        pos = (i + 1) * mb // (cache_k + 1)
                        order.insert(min(pos, len(order)), i)
                for oi, mbx in enumerate(order):
                    if mbx < cache_k:
                        att = at_cache[mbx]
                        if layer == 0:
                            nc.sync.dma_start(att[:], panel_src(mbx))
                    else:
                        att = atp.tile([P, tpb, n], dt_st, tag="att")
                        nc.sync.dma_start(att[:], panel_src(mbx))
                    for t in range(tpb):
                        mt = mbx * tpb + t
                        for ncx in range(nch):
                            nc.tensor.matmul(
                                ps_list[ncx][:],
                                mm(g_sb[:, mt * P:(mt + 1) * P]),
                                mm(att[:, t, ncx * free:(ncx + 1) * free]),
                                start=(oi == 0 and t == 0),
                                stop=(oi == len(order) - 1 and t == tpb - 1),
                            )
                for ncx in range(nch):
                    if last:
                        oc = outp.tile([C, free], f32, tag="oc")
                        nc.vector.tensor_copy(oc[:], ps_list[ncx][:])
                        nc.scalar.dma_start(
                            out_h[:, ncx * free:(ncx + 1) * free], oc[:]
                        )
                    else:
                        _leaky(
                            nc,
                            dest[:, ncx * free:(ncx + 1) * free],
                            ps_list[ncx],
                            lkp,
                            free,
                        )
                ht_cur = dest

    nc.compile()
    return nc


def _get_nc(n, free, dt_big_name):
    key = (n, free, dt_big_name)
    if key not in _CACHE:
        _CACHE[key] = _build_nc(n, free, dt_big_name)
    return _CACHE[key]


def _block_diag(w, reps):
    """(D,D) -> (reps*D, reps*D) block diagonal, f32."""
    d = w.shape[0]
    out = np.zeros((reps * d, reps * d), dtype=np.float32)
    for b in range(reps):
        out[b * d:(b + 1) * d, b * d:(b + 1) * d] = w
    return out


def prepare_inputs(x, adj, Identity, W0, W1, W2, n=N_FULL, dt_big_name="float32"):
    """Host-side layout prep. Returns per-core input maps."""
    b_full = x.shape[0]
    b_core = b_full // NCORES
    c = b_core * D

    if dt_big_name == "bfloat16":
        import ml_dtypes
        np_st = ml_dtypes.bfloat16
    elif dt_big_name == "float16":
        np_st = np.float16
    else:
        np_st = np.float32

    at = np.ascontiguousarray(
        adj.T.astype(np.float32) + Identity.T.astype(np.float32)
    ).astype(np_st)

    reps = c // D
    w_all = np.stack(
        [
            _block_diag(np.asarray(W0, np.float32), reps),
            _block_diag(np.asarray(W1, np.float32), reps),
            _block_diag(np.asarray(W2, np.float32), reps),
            np.eye(c, dtype=np.float32),
        ]
    ).astype(np_st)

    # xt[core][b*D+d, m] = x[core*b_core + b, m, d]
    xf = np.asarray(x, np.float32)
    in_maps = []
    for core in range(NCORES):
        xs = xf[core * b_core:(core + 1) * b_core]      # (b_core, n, D)
        xt = np.ascontiguousarray(xs.transpose(0, 2, 1).reshape(c, n)).astype(np_st)
        in_maps.append({"xt": xt, "at": at, "wt": w_all})
    return in_maps


def gather_output(results, n=N_FULL, b_full=B_FULL):
    b_core = b_full // NCORES
    c = b_core * D
    out = np.empty((b_full, n, D), dtype=np.float32)
    for core in range(NCORES):
        oc = np.asarray(results[core]["out"], np.float32).reshape(b_core, D, n)
        out[core * b_core:(core + 1) * b_core] = oc.transpose(0, 2, 1)
    return out


def run(x, adj, Identity, W0, W1, W2, n=N_FULL, free=512,
        dt_big_name="float16", trace=False):
    from concourse.bass_utils import run_bass_kernel_spmd

    nc = _get_nc(n, free, dt_big_name)
    in_maps = prepare_inputs(x, adj, Identity, W0, W1, W2, n, dt_big_name)
    core_ids = list(range(NCORES))
    res = run_bass_kernel_spmd(nc, in_maps, core_ids, trace=trace)
    out = gather_output(res.results, n, x.shape[0])
    return out, res


def kernel(x, adj, Identity, W0, W1, W2):
    out, _ = run(x, adj, Identity, W0, W1, W2)
    return out


# revision 23
# speedup vs baseline: 1.1529x; 1.0722x over previous
"""GCN block (3 layers) on 8 trn2 NeuronCores, data-parallel over batch.

Math: each layer is X' = (adj + I) @ leaky_relu(X @ W).
Let A = adj + I. Using (A @ H) @ W == A @ (H @ W), fold each layer's weight
into the previous layer's output so every layer is one big matmul against A:

    H0 = lrelu(X0 W0)                 (tiny, on-chip)
    G0 = H0 W1 ; Z1 = A G0 ; H1 = lrelu(Z1)
    G1 = H1 W2 ; Z2 = A G1 ; H2 = lrelu(Z2)
    G2 = H2     ; X3 = A G2           (final output)

Per core: 8 samples x 16 features = 128 = partition width. Layouts:
    T-layout  [c=(b,d), m]   (128 partitions, N free)
    N-layout  [m, c]         (m partitions, 128 free)
Big matmul: out = lhsT.T @ rhs with lhsT = G (N-layout, stationary) and
rhs = A^T tiles (streamed from HBM) -> Z^T in T-layout. The 16x16 weights
are expanded to 128x128 block-diagonal so the tiny matmuls run all 8
samples at once:  G = (H^T)^T @ W_blk  via lhsT = H^T tile (T-layout).
A^T = adj.T + I is built on the host (layout prep), streamed 3x per core.
"""

import numpy as np

N_FULL = 4096
D = 16
B_FULL = 64
NCORES = 8
B_CORE = B_FULL // NCORES  # 8
C = B_CORE * D  # 128 partitions
P = 128
NEG_SLOPE = 0.2

_CACHE = {}


def _leaky(nc, dest, ps, pool, width):
    """dest = leaky_relu(ps) = 0.2*ps + 0.8*relu(ps), PSUM -> SBUF.

    Split across engines: ACT computes t = relu(0.8*ps) (scale commutes
    with relu), DVE computes dest = ps*0.2 + t. Each instruction reads
    PSUM at most once (HW constraint).
    """
    import concourse.mybir as mybir

    t = pool.tile([P, width], mybir.dt.float32, tag="lk")
    nc.scalar.activation(
        t[:], ps[:], mybir.ActivationFunctionType.Relu, scale=1.0 - NEG_SLOPE
    )
    nc.vector.scalar_tensor_tensor(
        dest, ps[:], NEG_SLOPE, t[:], mybir.AluOpType.mult, mybir.AluOpType.add
    )


def _build_nc(n, free, dt_big_name):
    """Build the Bass module (per-core program). Cached per config.

    dt_big_name: "float32" | "bfloat16" | "float32r".
      bfloat16: A^T/G/H^T/X^T/W stored bf16 (half DMA, full-rate PE).
      float32r: f32 storage, matmuls bitcast to fp32r (full-rate PE at
                free>=256, fp32 DMA cost, ~tf32 matmul precision).
    """
    import concourse.bass as bass
    import concourse.mybir as mybir
    import concourse.tile as tile
    from concourse import bacc

    f32 = mybir.dt.float32
    r32 = dt_big_name == "float32r"
    dt_st = f32 if r32 else getattr(mybir.dt, dt_big_name)  # storage dtype
    dt_act = dt_st  # activations/weights storage

    def mm(ap):
        # matmul-operand view: bitcast to fp32r in r32 mode
        return ap.bitcast(mybir.dt.float32r) if r32 else ap

    nt = n // P        # number of 128-row m-tiles
    nch = n // free    # output column chunks
    tpb = 2              # m-tiles per A^T panel
    mb = n // (tpb * P)  # number of A^T row panels

    nc = bacc.Bacc(
        "TRN2", target_bir_lowering=False, debug=False, num_devices=NCORES
    )
    xt_h = nc.dram_tensor("xt", [C, n], dt_act, kind="ExternalInput")
    at_h = nc.dram_tensor("at", [n, n], dt_st, kind="ExternalInput")
    w_h = nc.dram_tensor("wt", [4, P, P], dt_act, kind="ExternalInput")
    out_h = nc.dram_tensor("out", [C, n], f32, kind="ExternalOutput")

    cache_k = 5 if mybir.dt.size(dt_st) == 2 else 0  # A^T panels pinned in SBUF
    cache_k = min(cache_k, mb)
    at_bufs = 4 if mybir.dt.size(dt_st) == 2 else 3

    def panel_src(i):
        return at_h[i * tpb * P:(i + 1) * tpb * P, :].rearrange(
            "(t p) n -> p t n", p=P
        )

    with tile.TileContext(nc) as tc:
        with (
            tc.tile_pool(name="const", bufs=1) as constp,
            tc.tile_pool(name="xtp", bufs=2) as xtp,
            tc.tile_pool(name="ht", bufs=2) as htp,
            tc.tile_pool(name="g", bufs=2) as gp,
            tc.tile_pool(name="ats", bufs=at_bufs) as atp,
            tc.tile_pool(name="outp", bufs=2) as outp,
            tc.tile_pool(name="lk", bufs=2) as lkp,
            tc.tile_pool(name="ps", bufs=8, space="PSUM") as psp,
        ):
            w_sb = constp.tile([P, 4, P], dt_act)
            nc.sync.dma_start(w_sb[:], w_h[:].rearrange("w p q -> p w q"))

            # resident A^T panels: filled during layer 0, reused by layers 1-2
            at_cache = [
                constp.tile([P, tpb, n], dt_st, name=f"atc{i}")
                for i in range(cache_k)
            ]

            # H0^T = lrelu(W0_blk.T @ X0^T)  (T-layout)
            ht_cur = htp.tile([C, n], dt_act)
            for ch in range(nch):
                xtc = xtp.tile([C, free], dt_act, tag="xtc")
                nc.sync.dma_start(xtc[:], xt_h[:, ch * free:(ch + 1) * free])
                ps = psp.tile([P, free], f32, tag="ps")
                nc.tensor.matmul(
                    ps[:], mm(w_sb[:, 0, :]), mm(xtc[:]), start=True, stop=True
                )
                _leaky(nc, ht_cur[:, ch * free:(ch + 1) * free], ps, lkp, free)

            for layer in range(3):
                w_idx = layer + 1  # W1_blk, W2_blk, I128
                # tiny: G = (H^T)^T @ W_blk  (N-layout)
                g_sb = gp.tile([P, n], dt_st)
                for mt in range(nt):
                    psg = psp.tile([P, P], f32, tag="ps")
                    nc.tensor.matmul(
                        psg[:],
                        mm(ht_cur[:, mt * P:(mt + 1) * P]),
                        mm(w_sb[:, w_idx, :]),
                        start=True,
                        stop=True,
                    )
                    nc.vector.tensor_copy(g_sb[:, mt * P:(mt + 1) * P], psg[:])

                # big: Z^T = sum_m G[m,:].T @ A^T[m, :]
                # m-outer: stream full row-panels of A^T (fat contiguous
                # DMA runs); all nch psum banks accumulate in parallel;
                # one stationary G tile serves nch matmuls per t-step.
                last = layer == 2
                dest = None if last else htp.tile([C, n], dt_act, name="htn")
                ps_list = [
                    psp.tile([P, free], f32, tag="ps", name=f"psc{i}")
                    for i in range(nch)
                ]
                # streamed panels with cached panels interleaved so the
                # stream prefetch catches up during DMA-free cached phases;
                # final layer runs cached panels first so the kernel ENDS
                # on streamed panels (DMA busy to the last matmul)
                streamed = list(range(cache_k, mb))
                if last:
                    order = list(range(cache_k)) + streamed
                else:
                    order = streamed[:]
                    for i in range(cache_k):
                        pos = (i + 1) * mb // (cache_k + 1)
                        order.insert(min(pos, len(order)), i)
                for oi, mbx in enumerate(order):
                    if mbx < cache_k:
                        att = at_cache[mbx]
                        if layer == 0:
                            nc.sync.dma_start(att[:], panel_src(mbx))
                    else:
                        att = atp.tile([P, tpb, n], dt_st, tag="att")
                        nc.sync.dma_start(att[:], panel_src(mbx))
                    for t in range(tpb):
                        mt = mbx * tpb + t
                        for ncx in range(nch):
                            nc.tensor.matmul(
                                ps_list[ncx][:],
                                mm(g_sb[:, mt * P:(mt + 1) * P]),
                                mm(att[:, t, ncx * free:(ncx + 1) * free]),
                                start=(oi == 0 and t == 0),
                                stop=(oi == len(order) - 1 and t == tpb - 1),
                            )
                for ncx in range(nch):
                    if last:
                        oc = outp.tile([C, free], f32, tag="oc")
                        nc.vector.tensor_copy(oc[:], ps_list[ncx][:])
                        nc.scalar.dma_start(
                            out_h[:, ncx * free:(ncx + 1) * free], oc[:]
                        )
                    else:
                        _leaky(
                            nc,
                            dest[:, ncx * free:(ncx + 1) * free],
                            ps_list[ncx],
                            lkp,
                            free,
                        )
                ht_cur = dest

    nc.compile()
    return nc


def _get_nc(n, free, dt_big_name):
    key = (n, free, dt_big_name)
    if key not in _CACHE:
        _CACHE[key] = _build_nc(n, free, dt_big_name)
    return _CACHE[key]


def _block_diag(w, reps):
    """(D,D) -> (reps*D, reps*D) block diagonal, f32."""
    d = w.shape[0]
    out = np.zeros((reps * d, reps * d), dtype=np.float32)
    for b in range(reps):
        out[b * d:(b + 1) * d, b * d:(b + 1) * d] = w
    return out


def prepare_inputs(x, adj, Identity, W0, W1, W2, n=N_FULL, dt_big_name="float32"):
    """Host-side layout prep. Returns per-core input maps."""
    b_full = x.shape[0]
    b_core = b_full // NCORES
    c = b_core * D

    if dt_big_name == "bfloat16":
        import ml_dtypes
        np_st = ml_dtypes.bfloat16
    elif dt_big_name == "float16":
        np_st = np.float16
    else:
        np_st = np.float32

    at = np.ascontiguousarray(
        adj.T.astype(np.float32) + Identity.T.astype(np.float32)
    ).astype(np_st)

    reps = c // D
    w_all = np.stack(
        [
            _block_diag(np.asarray(W0, np.float32), reps),
            _block_diag(np.asarray(W1, np.float32), reps),
            _block_diag(np.asarray(W2, np.float32), reps),
            np.eye(c, dtype=np.float32),
        ]
    ).astype(np_st)

    # xt[core][b*D+d, m] = x[core*b_core + b, m, d]
    xf = np.asarray(x, np.float32)
    in_maps = []
    for core in range(NCORES):
        xs = xf[core * b_core:(core + 1) * b_core]      # (b_core, n, D)
        xt = np.ascontiguousarray(xs.transpose(0, 2, 1).reshape(c, n)).astype(np_st)
        in_maps.append({"xt": xt, "at": at, "wt": w_all})
    return in_maps


def gather_output(results, n=N_FULL, b_full=B_FULL):
    b_core = b_full // NCORES
    c = b_core * D
    out = np.empty((b_full, n, D), dtype=np.float32)
    for core in range(NCORES):
        oc = np.asarray(results[core]["out"], np.float32).reshape(b_core, D, n)
        out[core * b_core:(core + 1) * b_core] = oc.transpose(0, 2, 1)
    return out


def run(x, adj, Identity, W0, W1, W2, n=N_FULL, free=512,
        dt_big_name="float16", trace=False):
    from concourse.bass_utils import run_bass_kernel_spmd

    nc = _get_nc(n, free, dt_big_name)
    in_maps = prepare_inputs(x, adj, Identity, W0, W1, W2, n, dt_big_name)
    core_ids = list(range(NCORES))
    res = run_bass_kernel_spmd(nc, in_maps, core_ids, trace=trace)
    out = gather_output(res.results, n, x.shape[0])
    return out, res


def kernel(x, adj, Identity, W0, W1, W2):
    out, _ = run(x, adj, Identity, W0, W1, W2)
    return out
